# revision 18
# baseline (speedup 1.0000x reference)
"""Cross-attention Trainium2 kernel (8 NeuronCores, Bass/Tile).

Problem (hardcoded): B=2, SQ=SKV=2048, D=1024, H=16 heads, HD=64.
  q  = query @ Wq + bq
  kv = context @ Wkv + bkv ; split into k, v per head
  o  = softmax(q k^T / sqrt(hd) + mask) v         (mask: -inf where True)
  out = o @ Wout + bout

Sharding: core c = (b, g) with b = c // 4 (batch), g = c % 4 (head group of 4).
Each core computes its batch's attention for its 4 heads and the partial out
projection (Wout rows for those heads); host sums the 4 partials per batch and
adds bout (linearity of the out projection).

Everything on-chip runs "transposed" (feature dim on partitions, tokens on the
free dim). All matmul operands are bfloat16 (fp32 PSUM accumulation): bf16
streams at 1 row/cycle on the PE unconditionally, gets fast weight loads, and
halves DMA/SBUF vs fp32r. Verified numerics: rel err ~6e-3 vs the fp32
reference (gate 2e-2).

Scores matmuls have K=HD=64, so a head pair (features at partitions 0-63 and
64-127 of the same band) occupies disjoint row halves of the 128x128 PE array.
Emitting the two heads' scores matmuls back-to-back lets the PE run them
concurrently (row-group tiling), ~2x on the scores third of attention.

Softmax uses no max subtraction (scores ~N(0,1); exp safe in fp32). Masking
folds into V: v rows scale by keep=1-mask and an extra keep column of V yields
the softmax denominator through the same PE accumulation. The reciprocal
denominators for both heads of a pair broadcast via one K=2 matmul.
"""

import sys

sys.path.insert(0, "/opt/trn_rl_repo")

import numpy as np

B, SQ, SKV, D, H, HD = 2, 2048, 2048, 1024, 16, 64
HG = 4                # heads per core
COLS = HG * HD        # 256 projected columns per core (per q/k/v)
DK = D // 128         # 8 contraction tiles
SQC = 512             # sq chunk (psum bank)
NSQC = SQ // SQC
SKC = 512             # skv chunk for kv projection
NSKC = SKV // SKC
NJ = SKV // 128       # 16 skv tiles for attention

_CACHE = {}


def _build(with_bias=False, reps=1):
    # reps>1 repeats the whole kernel body inside one NEFF (benchmark builds:
    # differencing two rep counts cancels dispatch overhead exactly)
    import os as _os
    pair = _os.environ.get("KERNEL_PAIR", "1") == "1"
    import concourse.bacc as bacc
    import concourse.mybir as mybir
    import concourse.tile as tile

    F32 = mybir.dt.float32
    F32R = mybir.dt.float32r
    BF16 = mybir.dt.bfloat16
    EXP = mybir.ActivationFunctionType.Exp

    nc = bacc.Bacc()

    # ---- DRAM I/O (per core) ----
    qryT = nc.dram_tensor("qryT", [D, SQ], BF16, kind="ExternalInput")
    ctxT = nc.dram_tensor("ctxT", [D, SKV], BF16, kind="ExternalInput")
    wq = nc.dram_tensor("wq", [D, COLS], BF16, kind="ExternalInput")
    wk = nc.dram_tensor("wk", [D, COLS], BF16, kind="ExternalInput")
    wv = nc.dram_tensor("wv", [D, COLS], BF16, kind="ExternalInput")
    wout = nc.dram_tensor("wout", [COLS, D], BF16, kind="ExternalInput")
    keep = nc.dram_tensor("keep", [128, NJ], F32, kind="ExternalInput")
    keepc = nc.dram_tensor("keepc", [128, NJ * HG], BF16, kind="ExternalInput")
    ones64 = nc.dram_tensor("ones64", [1, HD], F32R, kind="ExternalInput")
    if with_bias:
        bq = nc.dram_tensor("bq", [1, COLS], BF16, kind="ExternalInput")
        bk = nc.dram_tensor("bk", [1, COLS], BF16, kind="ExternalInput")
        bv = nc.dram_tensor("bv", [1, COLS], BF16, kind="ExternalInput")
        ones = nc.dram_tensor("ones", [1, SQC], BF16, kind="ExternalInput")
    outT = nc.dram_tensor("outT", [D, SQ], BF16, kind="ExternalOutput")

    with tile.TileContext(nc) as tc:
        with (
            tc.tile_pool(name="w", bufs=1) as wp,
            tc.tile_pool(name="big", bufs=1) as bigp,
            tc.tile_pool(name="strips", bufs=4) as sp,
            tc.tile_pool(name="work", bufs=1) as workp,
            tc.tile_pool(name="ps", bufs=1, space="PSUM") as psp,
        ):
            # ---- weights / constants ----
            wq_sb = wp.tile([128, DK, COLS], BF16)
            wk_sb = wp.tile([128, DK, COLS], BF16)
            wv_sb = wp.tile([128, DK, COLS], BF16)
            wout_sb = wp.tile([128, 2, D], BF16)
            keep_sb = wp.tile([128, NJ], F32)
            keepc_sb = wp.tile([128, NJ, HG], BF16)
            ones64_sb = wp.tile([1, HD], F32R)
            if with_bias:
                bq_sb = wp.tile([1, COLS], BF16)
                bk_sb = wp.tile([1, COLS], BF16)
                bv_sb = wp.tile([1, COLS], BF16)
                ones_sb = wp.tile([1, SQC], BF16)

            ctxT_r = ctxT.ap().rearrange("(t p) s -> p t s", p=128)
            qryT_r = qryT.ap().rearrange("(t p) s -> p t s", p=128)
            outT_r = outT.ap().rearrange("(t p) s -> p t s", p=128)


            # Benchmark builds repeat the whole body; weights re-DMA each
            # rep (DMA has ample headroom under the PE roofline).
            for _rep in range(reps):
                # ---- persistent activations ----
                kt_sb = bigp.tile([128, 2, SKV], BF16)         # k^T, head pair per 64-row band
                v_sb = bigp.tile([128, NJ, HG, HD + 1], BF16)  # v + keep column
                qt_all = bigp.tile([128, 2, SQ], BF16)         # q^T for all chunks

                # Startup-critical DMAs split per d-tile so the first kT matmul
                # (needs wk d=0 + ctx d=0 only) starts ASAP.
                wk_r = wk.ap().rearrange("(t p) m -> p t m", p=128)
                ctx0_sb = sp.tile([128, DK, SKC], BF16, tag="strip")
                nc.sync.dma_start(wk_sb[:, 0:1, :], wk_r[:, 0:1, :])
                nc.sync.dma_start(ctx0_sb[:, 0:2, :], ctxT_r[:, 0:2, 0:SKC])
                if with_bias:
                    nc.sync.dma_start(bk_sb[:], bk.ap())
                    nc.sync.dma_start(ones_sb[:], ones.ap())
                nc.sync.dma_start(wk_sb[:, 1:4, :], wk_r[:, 1:4, :])
                nc.sync.dma_start(ctx0_sb[:, 2:4, :], ctxT_r[:, 2:4, 0:SKC])
                nc.sync.dma_start(wk_sb[:, 4:8, :], wk_r[:, 4:8, :])
                nc.sync.dma_start(ctx0_sb[:, 4:6, :], ctxT_r[:, 4:6, 0:SKC])
                nc.sync.dma_start(ctx0_sb[:, 6:8, :], ctxT_r[:, 6:8, 0:SKC])
                # qproj(0) inputs right after the kT inputs.
                wq_r = wq.ap().rearrange("(t p) m -> p t m", p=128)
                qry0_sb = sp.tile([128, DK, SQC], BF16, tag="strip", name="qry0_sb")
                if with_bias:
                    nc.sync.dma_start(bq_sb[:], bq.ap())
                nc.scalar.dma_start(wq_sb[:, 0:4, :], wq_r[:, 0:4, :])
                nc.scalar.dma_start(qry0_sb[:, 0:2, :], qryT_r[:, 0:2, 0:SQC])
                nc.scalar.dma_start(wq_sb[:, 4:8, :], wq_r[:, 4:8, :])
                nc.scalar.dma_start(qry0_sb[:, 2:4, :], qryT_r[:, 2:4, 0:SQC])
                nc.scalar.dma_start(qry0_sb[:, 4:6, :], qryT_r[:, 4:6, 0:SQC])
                nc.scalar.dma_start(qry0_sb[:, 6:8, :], qryT_r[:, 6:8, 0:SQC])
                nc.gpsimd.dma_start(wv_sb[:], wv.ap().rearrange("(t p) m -> p t m", p=128))
                if with_bias:
                    nc.sync.dma_start(bv_sb[:], bv.ap())
                nc.gpsimd.dma_start(keep_sb[:], keep.ap())
                nc.gpsimd.dma_start(
                    keepc_sb[:], keepc.ap().rearrange("p (j h) -> p j h", h=HG)
                )
                nc.gpsimd.dma_start(ones64_sb[:], ones64.ap())
                # keep columns of V written once (v-proj fills columns 0:HD)
                nc.vector.tensor_copy(v_sb[:, :, :, HD:HD + 1], keepc_sb[:])
                # pre-issue the remaining ctx strips ahead of wout / later qry
                strip_tiles = [ctx0_sb]
                for jc in range(1, NSKC):
                    st = sp.tile([128, DK, SKC], BF16, tag="strip", name=f"ctx{jc}_sb")
                    nc.scalar.dma_start(st[:], ctxT_r[:, :, jc * SKC:(jc + 1) * SKC])
                    strip_tiles.append(st)

                # ============ kv-projection emitters ============
                def emit_K_kT(jc):
                    ctx_sb = strip_tiles[jc]
                    pk = psp.tile([128, 2, SKC], F32, tag="mm", bufs=2, name="pk")
                    for cc in range(2):
                        for d in range(DK):
                            nc.tensor.matmul(
                                pk[:, cc, :],
                                wk_sb[:, d, cc * 128:(cc + 1) * 128],
                                ctx_sb[:, d, :],
                                start=(d == 0), stop=(not with_bias and d == DK - 1),
                            )
                        if with_bias:
                            nc.tensor.matmul(
                                pk[:, cc, :],
                                bk_sb[0:1, cc * 128:(cc + 1) * 128],
                                ones_sb[0:1, :],
                                start=False, stop=True,
                            )
                    nc.vector.tensor_copy(kt_sb[:, :, jc * SKC:(jc + 1) * SKC], pk[:])

                def emit_K_v(jc, jjp):
                    ctx_sb = strip_tiles[jc]
                    pv = psp.tile([128, 2, SKC], F32, tag="mm", bufs=2, name="pv")
                    for sub in range(2):
                        jj = jjp * 2 + sub
                        for d in range(DK):
                            nc.tensor.matmul(
                                pv[:, sub, 0:COLS],
                                ctx_sb[:, d, jj * 128:(jj + 1) * 128],
                                wv_sb[:, d, :],
                                start=(d == 0), stop=(not with_bias and d == DK - 1),
                            )
                        if with_bias:
                            nc.tensor.matmul(
                                pv[:, sub, 0:COLS],
                                ones_sb[0:1, 0:128],
                                bv_sb[0:1, :],
                                start=False, stop=True,
                            )
                    for sub in range(2):
                        jj = jjp * 2 + sub
                        j = jc * 4 + jj
                        nc.vector.tensor_scalar_mul(
                            v_sb[:, j, :, 0:HD],
                            pv[:, sub, 0:COLS].rearrange("p (h e) -> p h e", h=HG),
                            keep_sb[:, j:j + 1],
                        )

                # ============ q / out projection filler generators ============
                def gen_qproj(qc, qry_sb=None):
                    if qry_sb is None:
                        qry_sb = sp.tile([128, DK, SQC], BF16, tag="strip", name="qry_sb")
                        nc.sync.dma_start(qry_sb[:], qryT_r[:, :, qc * SQC:(qc + 1) * SQC])
                    yield
                    for cc in range(2):
                        pq = psp.tile([128, SQC], F32, tag="av", bufs=2, name="pq")
                        for d in range(DK):
                            nc.tensor.matmul(
                                pq[:],
                                wq_sb[:, d, cc * 128:(cc + 1) * 128],
                                qry_sb[:, d, :],
                                start=(d == 0), stop=(not with_bias and d == DK - 1),
                            )
                            yield
                        if with_bias:
                            nc.tensor.matmul(
                                pq[:],
                                bq_sb[0:1, cc * 128:(cc + 1) * 128],
                                ones_sb[0:1, :],
                                start=False, stop=True,
                            )
                            yield
                        nc.vector.tensor_copy(
                            qt_all[:, cc, qc * SQC:(qc + 1) * SQC], pq[:]
                        )
                        yield

                def gen_outproj(qc, otn, epilogue=False):
                    for m in range(8):
                        pf = psp.tile([128, SQC], F32, tag="av", bufs=2, name="pf")
                        nc.tensor.matmul(
                            pf[:],
                            wout_sb[:, 0, m * 128:(m + 1) * 128],
                            otn[:, 0, :],
                            start=True, stop=False,
                        )
                        yield
                        nc.tensor.matmul(
                            pf[:],
                            wout_sb[:, 1, m * 128:(m + 1) * 128],
                            otn[:, 1, :],
                            start=False, stop=True,
                        )
                        yield
                        fin = workp.tile([128, SQC], BF16, tag="fin", bufs=4)
                        if epilogue and m % 2 == 0:
                            nc.scalar.copy(fin[:], pf[:])
                        else:
                            nc.vector.tensor_copy(fin[:], pf[:])
                        nc.sync.dma_start(
                            outT_r[:, m, qc * SQC:(qc + 1) * SQC], fin[:]
                        )
                        yield

                filler = []

                def emit_filler(budget):
                    while budget > 0 and filler:
                        try:
                            next(filler[0])
                            budget -= 1
                        except StopIteration:
                            filler.pop(0)

                # ============ attention block for one (qc, head pair cc) ============
                # Scores for the pair's two heads (K=64, partitions 0-63 / 64-127)
                # are emitted adjacently so the PE runs them concurrently in
                # disjoint row halves. AV (K=128, full array) lags two j-steps so
                # the PE never waits on a freshly issued exp.
                def attention_block(qc, cc, otn, kfeed=None):
                    ha, hb = 2 * cc, 2 * cc + 1
                    qt = qt_all[:, cc, qc * SQC:(qc + 1) * SQC]
                    pav_a = psp.tile([HD + 1, SQC], F32, tag="pav", bufs=2, name="pav_a")
                    pav_b = psp.tile([HD + 1, SQC], F32, tag="pav", bufs=2, name="pav_b")

                    def emit_av(item):
                        j, pt = item
                        nc.tensor.matmul(
                            pav_a[:], v_sb[:, j, ha, :], pt[:, 0, :],
                            start=(j == 0), stop=(j == NJ - 1),
                        )
                        nc.tensor.matmul(
                            pav_b[:], v_sb[:, j, hb, :], pt[:, 1, :],
                            start=(j == 0), stop=(j == NJ - 1),
                        )

                    def emit_scores(j, po):
                        nc.tensor.matmul(
                            ps2s[-1][:, po // 64, :],
                            kt_sb[po:po + 64, cc, j * 128:(j + 1) * 128],
                            qt[po:po + 64, :],
                            start=True, stop=True,
                        )

                    def emit_av_half(item, po):
                        j, pt = item
                        pav = pav_a if po == 0 else pav_b
                        h = ha if po == 0 else hb
                        nc.tensor.matmul(
                            pav[:], v_sb[:, j, h, :], pt[:, po // 64, :],
                            start=(j == 0), stop=(j == NJ - 1),
                        )

                    pending = []
                    ps2s = []
                    for j in range(NJ):
                        ps2s.append(psp.tile([128, 2, SQC], F32, tag="mm", bufs=2, name="ps2"))
                        drain = pending.pop(0) if len(pending) > 1 else None
                        if pair:
                            # adjacent K=64 scores pair -> concurrent row tiles
                            emit_scores(j, 0)
                            emit_scores(j, 64)
                            if drain is not None:
                                emit_av_half(drain, 0)
                                emit_av_half(drain, 64)
                        else:
                            # serial control: full-array AV between the pair
                            emit_scores(j, 0)
                            if drain is not None:
                                emit_av_half(drain, 0)
                            emit_scores(j, 64)
                            if drain is not None:
                                emit_av_half(drain, 64)
                        pt = workp.tile([128, 2, SQC], BF16, tag="pt", bufs=4)
                        nc.scalar.activation(pt[:], ps2s[-1][:], EXP)
                        pending.append((j, pt))
                        if kfeed is not None:
                            acts = kfeed.get(j, ())
                            for fn in acts:
                                fn()
                            if not acts:
                                emit_filler(1)
                        if kfeed is None:
                            emit_filler(2 if len(filler) > 1 else 1)
                    for item in pending:
                        emit_av_half(item, 0)
                        emit_av_half(item, 64)
                        emit_filler(1)

                    # normalize: divide by the keep-column accumulation.
                    # pav->ot copies + reciprocals run now (frees the psum
                    # banks); the broadcast + multiplies join the filler
                    # stream so the block boundary never stalls the PE.
                    ot_a = workp.tile([HD + 1, SQC], F32, tag="ot", bufs=4)
                    ot_b = workp.tile([HD + 1, SQC], F32, tag="ot", bufs=4)
                    nc.vector.tensor_copy(ot_a[:], pav_a[:])
                    nc.scalar.copy(ot_b[:], pav_b[:])
                    rcp_a = workp.tile([1, SQC], F32R, tag="rcp", bufs=4)
                    rcp_b = workp.tile([1, SQC], F32R, tag="rcp", bufs=4)
                    with nc.allow_low_precision(reason="fp32r reciprocal for softmax denom"):
                        nc.vector.reciprocal(rcp_a[:], ot_a[HD:HD + 1, :])
                        nc.vector.reciprocal(rcp_b[:], ot_b[HD:HD + 1, :])

                    def gen_norm():
                        pbc_a = psp.tile([128, SQC], F32, tag="av", bufs=2, name="pbc_a")
                        nc.tensor.matmul(
                            pbc_a[0:64, :], ones64_sb[0:1, :], rcp_a[:],
                            start=True, stop=True,
                        )
                        yield
                        pbc_b = psp.tile([128, SQC], F32, tag="av", bufs=2, name="pbc_b")
                        nc.tensor.matmul(
                            pbc_b[0:64, :], ones64_sb[0:1, :], rcp_b[:],
                            start=True, stop=True,
                        )
                        nc.vector.tensor_mul(otn[0:64, cc, :], ot_a[0:HD, :], pbc_a[0:64, :])
                        yield
                        nc.vector.tensor_mul(otn[64:128, cc, :], ot_b[0:HD, :], pbc_b[0:64, :])
                        yield

                    filler.insert(0, gen_norm())

                # ============ main schedule ============
                emit_K_kT(0)
                for _ in gen_qproj(0, qry0_sb):
                    pass

                def mk_kT(jc):
                    return lambda: emit_K_kT(jc)

                def mk_v(jc, jjp):
                    return lambda: emit_K_v(jc, jjp)

                def mk_wout_dma():
                    return lambda: nc.sync.dma_start(
                        wout_sb[:], wout.ap().rearrange("(t p) m -> p t m", p=128)
                    )

                # kv projection interleaved into the first attention block: kT(jc)
                # lands before scores j=4jc, v(jc) before AV j=4jc (AV lags 2).
                kfeed0 = {
                    0: (mk_v(0, 0), mk_v(0, 1)),
                    2: (mk_kT(1),),
                    4: (mk_v(1, 0),),
                    5: (mk_v(1, 1),),
                    6: (mk_kT(2),),
                    8: (mk_v(2, 0),),
                    9: (mk_v(2, 1), mk_wout_dma()),
                    10: (mk_kT(3),),
                    12: (mk_v(3, 0),),
                    13: (mk_v(3, 1),),
                }

                otn_prev = None
                for qc in range(NSQC):
                    otn = workp.tile([128, 2, SQC], BF16, tag="otn", bufs=2)
                    if qc + 1 < NSQC:
                        filler.append(gen_qproj(qc + 1))
                    if otn_prev is not None:
                        filler.append(gen_outproj(qc - 1, otn_prev))
                    attention_block(qc, 0, otn, kfeed=kfeed0 if qc == 0 else None)
                    attention_block(qc, 1, otn)
                    otn_prev = otn

                # drain remaining filler, then the final chunk's out-projection
                emit_filler(10 ** 9)
                for _ in gen_outproj(NSQC - 1, otn_prev, epilogue=True):
                    pass

    nc.compile()
    return nc


def _get_nc(with_bias=False, reps=1):
    import os as _os
    key = f"nc{int(with_bias)}r{reps}p{_os.environ.get('KERNEL_PAIR', '1')}"
    if key not in _CACHE:
        _CACHE[key] = _build(with_bias, reps=reps)
    return _CACHE[key]


LAST_RESULTS = None
LAST_IN_MAPS = None


def kernel(query, context, mask, Wq, bq, Wkv, bkv, Wout, bout, num_heads):
    import os
    import ml_dtypes
    from concourse.bass_utils import run_bass_kernel_spmd

    BF = ml_dtypes.bfloat16
    query = np.asarray(query, dtype=np.float32)
    context = np.asarray(context, dtype=np.float32)
    mask = np.asarray(mask)
    Wq = np.asarray(Wq, dtype=np.float32)
    bq_v = np.asarray(bq, dtype=np.float32)
    Wkv = np.asarray(Wkv, dtype=np.float32)
    bkv_v = np.asarray(bkv, dtype=np.float32)
    Wout = np.asarray(Wout, dtype=np.float32)
    bout_v = np.asarray(bout, dtype=np.float32)
    assert int(num_heads) == H

    scale = np.float32(HD ** -0.5)
    Wq_s = Wq * scale
    bq_s = bq_v * scale
    Wk = Wkv[:, :D]
    Wv = Wkv[:, D:]
    bk_v = bkv_v[:D]
    bv_v = bkv_v[D:]
    keep_f = 1.0 - mask.astype(np.float32)          # [B, SKV]
    ones64_m = np.ones((1, HD), dtype=np.float32)

    with_bias = bool(np.any(bq_s) or np.any(bk_v) or np.any(bv_v))
    nc = _get_nc(with_bias)
    in_maps = []
    for c in range(8):
        b, g = c // 4, c % 4
        cs = slice(g * COLS, (g + 1) * COLS)
        keep_b = np.ascontiguousarray(keep_f[b].reshape(NJ, 128).T)
        im = {
            "qryT": query[b].T.astype(BF),
            "ctxT": context[b].T.astype(BF),
            "wq": Wq_s[:, cs].astype(BF),
            "wk": Wk[:, cs].astype(BF),
            "wv": Wv[:, cs].astype(BF),
            "wout": Wout[cs, :].astype(BF),
            "keep": keep_b,
            "keepc": np.repeat(keep_b, HG, axis=1).astype(BF),
            "ones64": ones64_m,
        }
        if with_bias:
            im["bq"] = bq_s[cs][None, :].astype(BF)
            im["bk"] = bk_v[cs][None, :].astype(BF)
            im["bv"] = bv_v[cs][None, :].astype(BF)
            im["ones"] = np.ones((1, SQC), dtype=np.float32).astype(BF)
        in_maps.append(im)

    trace = bool(int(os.environ.get("KERNEL_TRACE", "0")))
    res = run_bass_kernel_spmd(nc, in_maps, core_ids=list(range(8)), trace=trace)
    global LAST_RESULTS, LAST_IN_MAPS
    LAST_RESULTS = res
    LAST_IN_MAPS = in_maps

    out = np.empty((B, SQ, D), dtype=np.float32)
    for b in range(B):
        acc = np.zeros((D, SQ), dtype=np.float32)
        for g in range(4):
            acc += res.results[b * 4 + g]["outT"].astype(np.float32)
        out[b] = acc.T + bout_v[None, :]
    return out



# revision 23
# speedup vs baseline: 1.5880x; 1.5880x over previous
"""Cross-attention Trainium2 kernel (8 NeuronCores, Bass/Tile).

Problem (hardcoded): B=2, SQ=SKV=2048, D=1024, H=16 heads, HD=64.
  q  = query @ Wq + bq
  kv = context @ Wkv + bkv ; split into k, v per head
  o  = softmax(q k^T / sqrt(hd) + mask) v         (mask: -inf where True)
  out = o @ Wout + bout

Sharding: core c = (b, g) with b = c // 4 (batch), g = c % 4 (head group of 4).
Each core computes its batch's attention for its 4 heads and the partial out
projection (Wout rows for those heads); host sums the 4 partials per batch and
adds bout (linearity of the out projection).

Everything on-chip runs "transposed" (feature dim on partitions, tokens on the
free dim). All matmul operands are bfloat16 (fp32 PSUM accumulation): bf16
streams at 1 row/cycle on the PE unconditionally, gets fast weight loads, and
halves DMA/SBUF vs fp32r. Verified numerics: rel err ~6e-3 vs the fp32
reference (gate 2e-2).

Scores matmuls have K=HD=64, so a head pair (features at partitions 0-63 and
64-127 of the same band) occupies disjoint row halves of the 128x128 PE array.
Emitting the two heads' scores matmuls back-to-back lets the PE run them
concurrently (row-group tiling), ~2x on the scores third of attention.

Softmax uses no max subtraction (scores ~N(0,1); exp safe in fp32). Masking
folds into V: v rows scale by keep=1-mask and an extra keep column of V yields
the softmax denominator through the same PE accumulation. The reciprocal
denominators for both heads of a pair broadcast via one K=2 matmul.
"""

import sys

sys.path.insert(0, "/opt/trn_rl_repo")

import numpy as np

B, SQ, SKV, D, H, HD = 2, 2048, 2048, 1024, 16, 64
HG = 4                # heads per core
COLS = HG * HD        # 256 projected columns per core (per q/k/v)
DK = D // 128         # 8 contraction tiles
SQC = 512             # sq chunk (psum bank)
NSQC = SQ // SQC
SKC = 512             # skv chunk for kv projection
NSKC = SKV // SKC
NJ = SKV // 128       # 16 skv tiles for attention

_CACHE = {}


def _build(with_bias=False, reps=1):
    # reps>1 repeats the whole kernel body inside one NEFF (benchmark builds:
    # differencing two rep counts cancels dispatch overhead exactly)
    import os as _os
    pair = _os.environ.get("KERNEL_PAIR", "1") == "1"
    import concourse.bacc as bacc
    import concourse.mybir as mybir
    import concourse.tile as tile

    F32 = mybir.dt.float32
    F32R = mybir.dt.float32r
    BF16 = mybir.dt.bfloat16
    EXP = mybir.ActivationFunctionType.Exp

    nc = bacc.Bacc()

    # ---- DRAM I/O (per core) ----
    qryT = nc.dram_tensor("qryT", [D, SQ], BF16, kind="ExternalInput")
    ctxT = nc.dram_tensor("ctxT", [D, SKV], BF16, kind="ExternalInput")
    wq = nc.dram_tensor("wq", [D, COLS], BF16, kind="ExternalInput")
    wk = nc.dram_tensor("wk", [D, COLS], BF16, kind="ExternalInput")
    wv = nc.dram_tensor("wv", [D, COLS], BF16, kind="ExternalInput")
    wout = nc.dram_tensor("wout", [COLS, D], BF16, kind="ExternalInput")
    keep = nc.dram_tensor("keep", [128, NJ], F32, kind="ExternalInput")
    keepc = nc.dram_tensor("keepc", [128, NJ * HG], BF16, kind="ExternalInput")
    ones64 = nc.dram_tensor("ones64", [1, HD], F32R, kind="ExternalInput")
    if with_bias:
        bq = nc.dram_tensor("bq", [1, COLS], BF16, kind="ExternalInput")
        bk = nc.dram_tensor("bk", [1, COLS], BF16, kind="ExternalInput")
        bv = nc.dram_tensor("bv", [1, COLS], BF16, kind="ExternalInput")
        ones = nc.dram_tensor("ones", [1, SQC], BF16, kind="ExternalInput")
    outT = nc.dram_tensor("outT", [D, SQ], BF16, kind="ExternalOutput")

    with tile.TileContext(nc) as tc:
        with (
            tc.tile_pool(name="w", bufs=1) as wp,
            tc.tile_pool(name="big", bufs=1) as bigp,
            tc.tile_pool(name="strips", bufs=4) as sp,
            tc.tile_pool(name="work", bufs=1) as workp,
            tc.tile_pool(name="ps", bufs=1, space="PSUM") as psp,
        ):
            # ---- weights / constants ----
            wq_sb = wp.tile([128, DK, COLS], BF16)
            wk_sb = wp.tile([128, DK, COLS], BF16)
            wv_sb = wp.tile([128, DK, COLS], BF16)
            wout_sb = wp.tile([128, 2, D], BF16)
            keep_sb = wp.tile([128, NJ], F32)
            keepc_sb = wp.tile([128, NJ, HG], BF16)
            ones64_sb = wp.tile([1, HD], F32R)
            if with_bias:
                bq_sb = wp.tile([1, COLS], BF16)
                bk_sb = wp.tile([1, COLS], BF16)
                bv_sb = wp.tile([1, COLS], BF16)
                ones_sb = wp.tile([1, SQC], BF16)

            ctxT_r = ctxT.ap().rearrange("(t p) s -> p t s", p=128)
            qryT_r = qryT.ap().rearrange("(t p) s -> p t s", p=128)
            outT_r = outT.ap().rearrange("(t p) s -> p t s", p=128)


            # Benchmark builds repeat the whole body; weights re-DMA each
            # rep (DMA has ample headroom under the PE roofline).
            for _rep in range(reps):
                # ---- persistent activations ----
                kt_sb = bigp.tile([128, 2, SKV], BF16)         # k^T, head pair per 64-row band
                v_sb = bigp.tile([128, NJ, HG, HD + 1], BF16)  # v + keep column
                qt_all = bigp.tile([128, 2, SQ], BF16)         # q^T for all chunks

                # Startup-critical DMAs split per d-tile so the first kT matmul
                # (needs wk d=0 + ctx d=0 only) starts ASAP.
                wk_r = wk.ap().rearrange("(t p) m -> p t m", p=128)
                ctx0_sb = sp.tile([128, DK, SKC], BF16, tag="strip")
                nc.sync.dma_start(wk_sb[:, 0:1, :], wk_r[:, 0:1, :])
                nc.sync.dma_start(ctx0_sb[:, 0:2, :], ctxT_r[:, 0:2, 0:SKC])
                if with_bias:
                    nc.sync.dma_start(bk_sb[:], bk.ap())
                    nc.sync.dma_start(ones_sb[:], ones.ap())
                nc.sync.dma_start(wk_sb[:, 1:4, :], wk_r[:, 1:4, :])
                nc.sync.dma_start(ctx0_sb[:, 2:4, :], ctxT_r[:, 2:4, 0:SKC])
                nc.sync.dma_start(wk_sb[:, 4:8, :], wk_r[:, 4:8, :])
                nc.gpsimd.dma_start(ctx0_sb[:, 4:6, :], ctxT_r[:, 4:6, 0:SKC])
                nc.gpsimd.dma_start(ctx0_sb[:, 6:8, :], ctxT_r[:, 6:8, 0:SKC])
                # qproj(0) inputs right after the kT inputs.
                wq_r = wq.ap().rearrange("(t p) m -> p t m", p=128)
                qry0_sb = sp.tile([128, DK, SQC], BF16, tag="strip", name="qry0_sb")
                if with_bias:
                    nc.sync.dma_start(bq_sb[:], bq.ap())
                nc.scalar.dma_start(wq_sb[:, 0:4, :], wq_r[:, 0:4, :])
                nc.scalar.dma_start(qry0_sb[:, 0:2, :], qryT_r[:, 0:2, 0:SQC])
                nc.scalar.dma_start(wq_sb[:, 4:8, :], wq_r[:, 4:8, :])
                nc.scalar.dma_start(qry0_sb[:, 2:4, :], qryT_r[:, 2:4, 0:SQC])
                nc.scalar.dma_start(qry0_sb[:, 4:6, :], qryT_r[:, 4:6, 0:SQC])
                nc.scalar.dma_start(qry0_sb[:, 6:8, :], qryT_r[:, 6:8, 0:SQC])
                nc.gpsimd.dma_start(wv_sb[:], wv.ap().rearrange("(t p) m -> p t m", p=128))
                if with_bias:
                    nc.sync.dma_start(bv_sb[:], bv.ap())
                nc.gpsimd.dma_start(keep_sb[:], keep.ap())
                nc.gpsimd.dma_start(
                    keepc_sb[:], keepc.ap().rearrange("p (j h) -> p j h", h=HG)
                )
                nc.gpsimd.dma_start(ones64_sb[:], ones64.ap())
                # keep columns of V written once (v-proj fills columns 0:HD)
                nc.vector.tensor_copy(v_sb[:, :, :, HD:HD + 1], keepc_sb[:])
                # pre-issue the remaining ctx strips ahead of wout / later qry
                strip_tiles = [ctx0_sb]
                strip_q = [nc.sync, nc.sync, nc.gpsimd]
                for jc in range(1, NSKC):
                    st = sp.tile([128, DK, SKC], BF16, tag="strip", name=f"ctx{jc}_sb")
                    strip_q[jc - 1].dma_start(st[:], ctxT_r[:, :, jc * SKC:(jc + 1) * SKC])
                    strip_tiles.append(st)

                # ============ kv-projection emitters ============
                def emit_K_kT(jc):
                    ctx_sb = strip_tiles[jc]
                    pk = psp.tile([128, 2, SKC], F32, tag="mm", bufs=2, name="pk")
                    for cc in range(2):
                        for d in range(DK):
                            nc.tensor.matmul(
                                pk[:, cc, :],
                                wk_sb[:, d, cc * 128:(cc + 1) * 128],
                                ctx_sb[:, d, :],
                                start=(d == 0), stop=(not with_bias and d == DK - 1),
                            )
                        if with_bias:
                            nc.tensor.matmul(
                                pk[:, cc, :],
                                bk_sb[0:1, cc * 128:(cc + 1) * 128],
                                ones_sb[0:1, :],
                                start=False, stop=True,
                            )
                    nc.vector.tensor_copy(kt_sb[:, :, jc * SKC:(jc + 1) * SKC], pk[:])

                def emit_K_v(jc, jjp):
                    ctx_sb = strip_tiles[jc]
                    pv = psp.tile([128, 2, SKC], F32, tag="mm", bufs=2, name="pv")
                    for sub in range(2):
                        jj = jjp * 2 + sub
                        for d in range(DK):
                            nc.tensor.matmul(
                                pv[:, sub, 0:COLS],
                                ctx_sb[:, d, jj * 128:(jj + 1) * 128],
                                wv_sb[:, d, :],
                                start=(d == 0), stop=(not with_bias and d == DK - 1),
                            )
                        if with_bias:
                            nc.tensor.matmul(
                                pv[:, sub, 0:COLS],
                                ones_sb[0:1, 0:128],
                                bv_sb[0:1, :],
                                start=False, stop=True,
                            )
                    for sub in range(2):
                        jj = jjp * 2 + sub
                        j = jc * 4 + jj
                        nc.vector.tensor_scalar_mul(
                            v_sb[:, j, :, 0:HD],
                            pv[:, sub, 0:COLS].rearrange("p (h e) -> p h e", h=HG),
                            keep_sb[:, j:j + 1],
                        )

                # ============ q / out projection filler generators ============
                def gen_qproj(qc, qry_sb=None):
                    if qry_sb is None:
                        qry_sb = sp.tile([128, DK, SQC], BF16, tag="strip", name="qry_sb")
                        nc.sync.dma_start(qry_sb[:], qryT_r[:, :, qc * SQC:(qc + 1) * SQC])
                    yield
                    for cc in range(2):
                        pq = psp.tile([128, SQC], F32, tag="av", bufs=2, name="pq")
                        for d in range(DK):
                            nc.tensor.matmul(
                                pq[:],
                                wq_sb[:, d, cc * 128:(cc + 1) * 128],
                                qry_sb[:, d, :],
                                start=(d == 0), stop=(not with_bias and d == DK - 1),
                            )
                            yield
                        if with_bias:
                            nc.tensor.matmul(
                                pq[:],
                                bq_sb[0:1, cc * 128:(cc + 1) * 128],
                                ones_sb[0:1, :],
                                start=False, stop=True,
                            )
                            yield
                        nc.vector.tensor_copy(
                            qt_all[:, cc, qc * SQC:(qc + 1) * SQC], pq[:]
                        )
                        yield

                def gen_outproj(qc, otn, epilogue=False):
                    for m in range(8):
                        pf = psp.tile([128, SQC], F32, tag="av", bufs=2, name="pf")
                        nc.tensor.matmul(
                            pf[:],
                            wout_sb[:, 0, m * 128:(m + 1) * 128],
                            otn[:, 0, :],
                            start=True, stop=False,
                        )
                        yield
                        nc.tensor.matmul(
                            pf[:],
                            wout_sb[:, 1, m * 128:(m + 1) * 128],
                            otn[:, 1, :],
                            start=False, stop=True,
                        )
                        yield
                        fin = workp.tile([128, SQC], BF16, tag="fin", bufs=4)
                        if epilogue and m % 2 == 0:
                            nc.scalar.copy(fin[:], pf[:])
                        else:
                            nc.vector.tensor_copy(fin[:], pf[:])
                        nc.sync.dma_start(
                            outT_r[:, m, qc * SQC:(qc + 1) * SQC], fin[:]
                        )
                        yield

                filler = []

                def emit_filler(budget):
                    while budget > 0 and filler:
                        try:
                            next(filler[0])
                            budget -= 1
                        except StopIteration:
                            filler.pop(0)

                # ============ attention block for one (qc, head pair cc) ============
                # Scores for the pair's two heads (K=64, partitions 0-63 / 64-127)
                # are emitted adjacently so the PE runs them concurrently in
                # disjoint row halves. AV (K=128, full array) lags two j-steps so
                # the PE never waits on a freshly issued exp.
                def attention_block(qc, cc, otn, kfeed=None):
                    ha, hb = 2 * cc, 2 * cc + 1
                    qt = qt_all[:, cc, qc * SQC:(qc + 1) * SQC]
                    pav_a = psp.tile([HD + 1, SQC], F32, tag="pav", bufs=2, name="pav_a")
                    pav_b = psp.tile([HD + 1, SQC], F32, tag="pav", bufs=2, name="pav_b")

                    def emit_av(item):
                        j, pt = item
                        nc.tensor.matmul(
                            pav_a[:], v_sb[:, j, ha, :], pt[:, 0, :],
                            start=(j == 0), stop=(j == NJ - 1),
                        )
                        nc.tensor.matmul(
                            pav_b[:], v_sb[:, j, hb, :], pt[:, 1, :],
                            start=(j == 0), stop=(j == NJ - 1),
                        )

                    def emit_scores(j, po):
                        nc.tensor.matmul(
                            ps2s[-1][:, po // 64, :],
                            kt_sb[po:po + 64, cc, j * 128:(j + 1) * 128],
                            qt[po:po + 64, :],
                            start=True, stop=True,
                        )

                    def emit_av_half(item, po):
                        j, pt = item
                        pav = pav_a if po == 0 else pav_b
                        h = ha if po == 0 else hb
                        nc.tensor.matmul(
                            pav[:], v_sb[:, j, h, :], pt[:, po // 64, :],
                            start=(j == 0), stop=(j == NJ - 1),
                        )

                    pending = []
                    ps2s = []
                    for j in range(NJ):
                        ps2s.append(psp.tile([128, 2, SQC], F32, tag="mm", bufs=2, name="ps2"))
                        drain = pending.pop(0) if len(pending) > 1 else None
                        if pair:
                            # adjacent K=64 scores pair -> concurrent row tiles
                            emit_scores(j, 0)
                            emit_scores(j, 64)
                            if drain is not None:
                                emit_av_half(drain, 0)
                                emit_av_half(drain, 64)
                        else:
                            # serial control: full-array AV between the pair
                            emit_scores(j, 0)
                            if drain is not None:
                                emit_av_half(drain, 0)
                            emit_scores(j, 64)
                            if drain is not None:
                                emit_av_half(drain, 64)
                        pt = workp.tile([128, 2, SQC], BF16, tag="pt", bufs=4)
                        nc.scalar.activation(pt[:], ps2s[-1][:], EXP)
                        pending.append((j, pt))
                        if kfeed is not None:
                            acts = kfeed.get(j, ())
                            for fn in acts:
                                fn()
                            if not acts:
                                emit_filler(1)
                        if kfeed is None:
                            emit_filler(2 if len(filler) > 1 else 1)
                    for item in pending:
                        emit_av_half(item, 0)
                        emit_av_half(item, 64)
                        emit_filler(1)

                    # normalize: divide by the keep-column accumulation.
                    # pav->ot copies + reciprocals run now (frees the psum
                    # banks); the broadcast + multiplies join the filler
                    # stream so the block boundary never stalls the PE.
                    ot_a = workp.tile([HD + 1, SQC], F32, tag="ot", bufs=4)
                    ot_b = workp.tile([HD + 1, SQC], F32, tag="ot", bufs=4)
                    nc.vector.tensor_copy(ot_a[:], pav_a[:])
                    nc.scalar.copy(ot_b[:], pav_b[:])
                    rcp_a = workp.tile([1, SQC], F32R, tag="rcp", bufs=4)
                    rcp_b = workp.tile([1, SQC], F32R, tag="rcp", bufs=4)
                    with nc.allow_low_precision(reason="fp32r reciprocal for softmax denom"):
                        nc.vector.reciprocal(rcp_a[:], ot_a[HD:HD + 1, :])
                        nc.vector.reciprocal(rcp_b[:], ot_b[HD:HD + 1, :])

                    def gen_norm():
                        pbc_a = psp.tile([128, SQC], F32, tag="av", bufs=2, name="pbc_a")
                        nc.tensor.matmul(
                            pbc_a[0:64, :], ones64_sb[0:1, :], rcp_a[:],
                            start=True, stop=True,
                        )
                        yield
                        pbc_b = psp.tile([128, SQC], F32, tag="av", bufs=2, name="pbc_b")
                        nc.tensor.matmul(
                            pbc_b[0:64, :], ones64_sb[0:1, :], rcp_b[:],
                            start=True, stop=True,
                        )
                        nc.vector.tensor_mul(otn[0:64, cc, :], ot_a[0:HD, :], pbc_a[0:64, :])
                        yield
                        nc.vector.tensor_mul(otn[64:128, cc, :], ot_b[0:HD, :], pbc_b[0:64, :])
                        yield

                    filler.insert(0, gen_norm())

                # ============ main schedule ============
                emit_K_kT(0)
                for _ in gen_qproj(0, qry0_sb):
                    pass

                def mk_kT(jc):
                    return lambda: emit_K_kT(jc)

                def mk_v(jc, jjp):
                    return lambda: emit_K_v(jc, jjp)

                def mk_wout_dma():
                    return lambda: nc.sync.dma_start(
                        wout_sb[:], wout.ap().rearrange("(t p) m -> p t m", p=128)
                    )

                # kv projection interleaved into the first attention block: kT(jc)
                # lands before scores j=4jc, v(jc) before AV j=4jc (AV lags 2).
                kfeed0 = {
                    0: (mk_v(0, 0), mk_v(0, 1)),
                    2: (mk_kT(1),),
                    4: (mk_v(1, 0),),
                    5: (mk_v(1, 1),),
                    6: (mk_kT(2),),
                    8: (mk_v(2, 0),),
                    9: (mk_v(2, 1), mk_wout_dma()),
                    10: (mk_kT(3),),
                    12: (mk_v(3, 0),),
                    13: (mk_v(3, 1),),
                }

                otn_prev = None
                for qc in range(NSQC):
                    otn = workp.tile([128, 2, SQC], BF16, tag="otn", bufs=2)
                    if qc + 1 < NSQC:
                        filler.append(gen_qproj(qc + 1))
                    if otn_prev is not None:
                        filler.append(gen_outproj(qc - 1, otn_prev))
                    attention_block(qc, 0, otn, kfeed=kfeed0 if qc == 0 else None)
                    attention_block(qc, 1, otn)
                    otn_prev = otn

                # drain remaining filler, then the final chunk's out-projection
                emit_filler(10 ** 9)
                for _ in gen_outproj(NSQC - 1, otn_prev, epilogue=True):
                    pass

    nc.compile()
    return nc


def _get_nc(with_bias=False, reps=1):
    import os as _os
    key = f"nc{int(with_bias)}r{reps}p{_os.environ.get('KERNEL_PAIR', '1')}"
    if key not in _CACHE:
        _CACHE[key] = _build(with_bias, reps=reps)
    return _CACHE[key]


LAST_RESULTS = None
LAST_IN_MAPS = None


def kernel(query, context, mask, Wq, bq, Wkv, bkv, Wout, bout, num_heads):
    import os
    import ml_dtypes
    from concourse.bass_utils import run_bass_kernel_spmd

    BF = ml_dtypes.bfloat16
    query = np.asarray(query, dtype=np.float32)
    context = np.asarray(context, dtype=np.float32)
    mask = np.asarray(mask)
    Wq = np.asarray(Wq, dtype=np.float32)
    bq_v = np.asarray(bq, dtype=np.float32)
    Wkv = np.asarray(Wkv, dtype=np.float32)
    bkv_v = np.asarray(bkv, dtype=np.float32)
    Wout = np.asarray(Wout, dtype=np.float32)
    bout_v = np.asarray(bout, dtype=np.float32)
    assert int(num_heads) == H

    scale = np.float32(HD ** -0.5)
    Wq_s = Wq * scale
    bq_s = bq_v * scale
    Wk = Wkv[:, :D]
    Wv = Wkv[:, D:]
    bk_v = bkv_v[:D]
    bv_v = bkv_v[D:]
    keep_f = 1.0 - mask.astype(np.float32)          # [B, SKV]
    ones64_m = np.ones((1, HD), dtype=np.float32)

    with_bias = bool(np.any(bq_s) or np.any(bk_v) or np.any(bv_v))
    nc = _get_nc(with_bias)
    in_maps = []
    for c in range(8):
        b, g = c // 4, c % 4
        cs = slice(g * COLS, (g + 1) * COLS)
        keep_b = np.ascontiguousarray(keep_f[b].reshape(NJ, 128).T)
        im = {
            "qryT": query[b].T.astype(BF),
            "ctxT": context[b].T.astype(BF),
            "wq": Wq_s[:, cs].astype(BF),
            "wk": Wk[:, cs].astype(BF),
            "wv": Wv[:, cs].astype(BF),
            "wout": Wout[cs, :].astype(BF),
            "keep": keep_b,
            "keepc": np.repeat(keep_b, HG, axis=1).astype(BF),
            "ones64": ones64_m,
        }
        if with_bias:
            im["bq"] = bq_s[cs][None, :].astype(BF)
            im["bk"] = bk_v[cs][None, :].astype(BF)
            im["bv"] = bv_v[cs][None, :].astype(BF)
            im["ones"] = np.ones((1, SQC), dtype=np.float32).astype(BF)
        in_maps.append(im)

    trace = bool(int(os.environ.get("KERNEL_TRACE", "0")))
    res = run_bass_kernel_spmd(nc, in_maps, core_ids=list(range(8)), trace=trace)
    global LAST_RESULTS, LAST_IN_MAPS
    LAST_RESULTS = res
    LAST_IN_MAPS = in_maps

    out = np.empty((B, SQ, D), dtype=np.float32)
    for b in range(B):
        acc = np.zeros((D, SQ), dtype=np.float32)
        for g in range(4):
            acc += res.results[b * 4 + g]["outT"].astype(np.float32)
        out[b] = acc.T + bout_v[None, :]
    return out

